# revision 23
# baseline (speedup 1.0000x reference)
"""Multi-head attention kernel for Trainium2 (Bass/Tile), 8 NeuronCores.

Problem: nn_MultiHeadAttention  (B=4, S=2048, D=1024, H=16, DK=64)
    out = softmax((q Wq^T + bq)(k Wk^T + bk)^T / sqrt(DK)) (v Wv^T + bv) Wo^T + bo

Sharding: core c = 2*b + g handles batch b and head-group g (8 heads = 512
features).  Each core computes its batch's attention for its heads plus a
partial output projection; the host sums the two partials per batch.

Math simplifications done on the host (exact):
  - k-bias bk drops out (adds a per-row constant to the logits; softmax is
    shift invariant along the key axis).
  - v-bias bv contributes softmax(S) @ (1 bv^T) Wo^T = bv^T Wo^T per row, a
    constant folded into an effective output bias bo_eff = bo + Wo @ bv.
  - the 1/sqrt(DK) logit scale is folded into Wq/bq.

On-chip layout (per core):
  - inputs are fed pre-transposed: qT/kT/vT [D, S] fp32, weights as W^T
    slices [D, 512] fp32 (wo as [512, D] bf16).
  - Q^T, K^T computed in [feature, seq] layout; V in natural [seq, feature]
    layout with a ones-column appended (gives the softmax denominator for
    free during the P@V matmul).
  - S^T = K_h Q_h^T per 128-row key chunk -> exp on ScalarE (PSUM->SBUF,
    bf16) -> O^T_unnorm = V^T E accumulated in PSUM; row 64 is the softmax
    denominator; reciprocal + gpsimd partition-broadcast + multiply
    normalizes.
  - output projection consumes O^T directly; host transposes the result.
"""

import numpy as np
import ml_dtypes
from contextlib import ExitStack

import concourse.bass as bass
import concourse.tile as tile
from concourse import bacc, mybir
from concourse.bass import ts, ds
from concourse.bass_utils import run_bass_kernel_spmd

B, S, D, H, DK = 4, 2048, 1024, 16, 64
N_CORES = 8
F32 = mybir.dt.float32
BF16 = mybir.dt.bfloat16
AF = mybir.ActivationFunctionType
ALU = mybir.AluOpType
BF16NP = ml_dtypes.bfloat16


def build_nc(s: int = S):
    """Build + compile the per-core Bass module (SPMD: same NEFF, per-core data).

    s: sequence length (parametric so the simulator self-test can run small).
    """
    assert s % 512 == 0
    nsi = s // 128   # 128-row key chunks
    nf = s // 512    # 512-col query chunks

    nc = bacc.Bacc("TRN2", target_bir_lowering=False, debug=False)

    qT = nc.dram_tensor("qT", [D, s], BF16, kind="ExternalInput").ap()
    kT = nc.dram_tensor("kT", [D, s], BF16, kind="ExternalInput").ap()
    vT = nc.dram_tensor("vT", [D, s], BF16, kind="ExternalInput").ap()
    wq = nc.dram_tensor("wq", [D, 512], BF16, kind="ExternalInput").ap()
    wk = nc.dram_tensor("wk", [D, 512], BF16, kind="ExternalInput").ap()
    wv = nc.dram_tensor("wv", [D, 512], BF16, kind="ExternalInput").ap()
    wo = nc.dram_tensor("wo", [512, D], BF16, kind="ExternalInput").ap()
    bq = nc.dram_tensor("bq", [128, 4], F32, kind="ExternalInput").ap()
    outT = nc.dram_tensor("outT", [D, s], F32, kind="ExternalOutput").ap()

    with tile.TileContext(nc) as tc, ExitStack() as ctx:
        pers = ctx.enter_context(tc.tile_pool(name="pers", bufs=1))
        pspool = ctx.enter_context(tc.tile_pool(name="ps", bufs=4, space="PSUM"))
        pss = ctx.enter_context(tc.tile_pool(name="pss", bufs=2, space="PSUM"))

        QT = pers.tile([128, 4, s], BF16)        # Q'^T  [feature, seq]
        KT = pers.tile([128, 4, s], BF16)        # K^T   [feature, seq]
        V = pers.tile([128, nsi, 8, 66], BF16)  # V nat [seq, head, dv|ones|pad]
        O = pers.tile([128, 4, s], BF16)        # O^T normalized
        WO = pers.tile([128, 4, D], BF16)
        BQ = pers.tile([128, 4], F32)

        nc.sync.dma_start(WO[:], wo.rearrange("(o p) e -> p o e", p=128))
        nc.sync.dma_start(BQ[:], bq)
        nc.vector.memset(V[:, :, :, 64:65], 1.0)

        # ---- phase 1: projections -------------------------------------
        epool = ctx.enter_context(tc.tile_pool(name="e", bufs=40))
        ph1 = ExitStack()
        xpool = ph1.enter_context(tc.tile_pool(name="x", bufs=16))
        wpool = ph1.enter_context(tc.tile_pool(name="w", bufs=1))
        for xdram, wdram, dst, bias in ((qT, wq, QT, BQ), (kT, wk, KT, None)):
            wt = wpool.tile([128, 8, 512], BF16, tag="w")
            nc.sync.dma_start(wt[:], wdram.rearrange("(o p) m -> p o m", p=128))
            for f in range(nf):
                xts = []
                for ki in range(8):
                    xt = xpool.tile([128, 512], BF16, tag="x")
                    nc.sync.dma_start(
                        xt[:], xdram[ds(ki * 128, 128), ds(f * 512, 512)]
                    )
                    xts.append(xt)
                for pc in range(4):
                    ps = pspool.tile([128, 512], F32, tag="ps")
                    for ki in range(8):
                        nc.tensor.matmul(
                            ps[:],
                            lhsT=wt[:, ki, ts(pc, 128)],
                            rhs=xts[ki][:],
                            start=(ki == 0),
                            stop=(ki == 7),
                        )
                    if bias is not None:
                        nc.vector.tensor_scalar_add(
                            dst[:, pc, ts(f, 512)], ps[:], bias[:, pc : pc + 1]
                        )
                    else:
                        nc.vector.tensor_copy(dst[:, pc, ts(f, 512)], ps[:])

        # Attention helpers -------------------------------------------
        hw_ = min(1024, s)      # S^T psum tile width (2 PSUM banks fp32)
        fph = hw_ // 512        # 512-col f-chunks per psum tile

        def qk_head(h):
            """S^T = K_h Q_h^T per key chunk, exp'ed into SBUF bf16 tiles."""
            hp, hh = h // 2, (h % 2) * 64
            es = []
            for si in range(nsi):
                halves = []
                for half in range(s // hw_):
                    ps = pss.tile([128, hw_], F32, tag="s", name=f"s_{h}_{si}_{half}")
                    for fo in range(fph):
                        f = fph * half + fo
                        nc.tensor.matmul(
                            ps[:, ts(fo, 512)],
                            lhsT=KT[ds(hh, 64), hp, ts(si, 128)],
                            rhs=QT[ds(hh, 64), hp, ts(f, 512)],
                            start=True,
                            stop=True,
                        )
                    e = epool.tile([128, hw_], BF16, tag="e", name=f"e_{h}_{si}_{half}")
                    nc.scalar.activation(e[:], ps[:], AF.Exp)
                    halves.append(e)
                es.append(halves)
            return es

        def pv_head(h, es):
            """O^T_unnorm = V_h^T E (ones col -> denom row 64), normalize."""
            hp, hh = h // 2, (h % 2) * 64
            pos = [
                pspool.tile([128, 512], F32, tag="ps", name=f"pos_{h}_{i}")
                for i in range(nf)
            ]
            for si in range(nsi):
                for f in range(nf):
                    nc.tensor.matmul(
                        pos[f][0:65, :],
                        lhsT=V[:, si, h, 0:65],
                        rhs=es[si][f // fph][:, ts(f % fph, 512)],
                        start=(si == 0),
                        stop=(si == nsi - 1),
                    )
            # Copy O_unnorm^T + denom row out of PSUM right away so the four
            # accumulator banks free for the next head's PV matmuls; the
            # normalize tail (recip + broadcast DMA + multiply) runs on SBUF.
            ou = oupool.tile([65, s], F32, tag="ou", name=f"ou_{h}")
            for f in range(nf):
                nc.vector.tensor_copy(ou[:, ts(f, 512)], pos[f][0:65, :])
            # row 64 of bsb holds 1/denom; replicate to rows 0-63 via a
            # DRAM-bounce broadcast DMA (step-0 partition AP on the read).
            bsb = bpool.tile([65, s], F32, tag="bsb", name=f"bsb_{h}")
            nc.vector.reciprocal(bsb[ds(64, 1), :], ou[ds(64, 1), :])
            dscr = dpool.tile([1, s], F32, tag="dscr", name=f"dscr_{h}")
            nc.sync.dma_start(dscr[:], bsb[ds(64, 1), :])
            nc.sync.dma_start(bsb[0:64, :], dscr[:].to_broadcast((64, s)))
            for f in range(nf):
                nc.vector.tensor_tensor(
                    O[ds(hh, 64), hp, ts(f, 512)],
                    ou[0:64, ts(f, 512)],
                    bsb[0:64, ts(f, 512)],
                    ALU.mult,
                )

        # Head 0's QK/exp is traced before the V projection so ScalarE gets
        # work as soon as the Q/K projections land.
        es0 = qk_head(0)

        # V projection: activations stationary so output lands [seq, feature]
        wt = wpool.tile([128, 8, 512], BF16, tag="w")
        nc.sync.dma_start(wt[:], wv.rearrange("(o p) m -> p o m", p=128))
        for f in range(nf):
            xts = []
            for ki in range(8):
                xt = xpool.tile([128, 512], BF16, tag="x")
                nc.sync.dma_start(xt[:], vT[ds(ki * 128, 128), ds(f * 512, 512)])
                xts.append(xt)
            for sj in range(4):
                si = f * 4 + sj
                ps = pspool.tile([128, 512], F32, tag="ps")
                for ki in range(8):
                    nc.tensor.matmul(
                        ps[:],
                        lhsT=xts[ki][:, ts(sj, 128)],
                        rhs=wt[:, ki, :],
                        start=(ki == 0),
                        stop=(ki == 7),
                    )
                nc.vector.tensor_copy(
                    V[:, si, :, 0:64], ps[:].rearrange("p (h d) -> p h d", h=8)
                )
        ph1.close()

        # ---- phase 2: remaining heads ---------------------------------
        ph2b = ExitStack()
        bpool = ph2b.enter_context(tc.tile_pool(name="b", bufs=2))
        oupool = ph2b.enter_context(tc.tile_pool(name="ou", bufs=2))
        dpool = ph2b.enter_context(tc.tile_pool(name="dscr", bufs=2, space="DRAM"))
        pv_head(0, es0)
        for h in range(1, 8):
            es = qk_head(h)
            pv_head(h, es)
        ph2b.close()

        # ---- phase 3: output projection (partial over this core's heads)
        opool = ctx.enter_context(tc.tile_pool(name="ostage", bufs=3))
        outr = outT.rearrange("(o p) n -> p o n", p=128)
        for pe in range(8):
            for f in range(nf):
                ps = pspool.tile([128, 512], F32, tag="ps")
                for ki in range(4):
                    nc.tensor.matmul(
                        ps[:],
                        lhsT=WO[:, ki, ts(pe, 128)],
                        rhs=O[:, ki, ts(f, 512)],
                        start=(ki == 0),
                        stop=(ki == 3),
                    )
                ot = opool.tile([128, 512], F32, tag="ot")
                nc.vector.tensor_copy(ot[:], ps[:])
                nc.sync.dma_start(outr[:, pe, ts(f, 512)], ot[:])

    nc.compile()
    return nc


_NC_CACHE: dict = {}


def get_nc(s: int = S):
    if s not in _NC_CACHE:
        _NC_CACHE[s] = build_nc(s)
    return _NC_CACHE[s]


def _prep_in_maps(q, k, v, Wq, bq, Wk, Wv, Wo):
    """Host-side shard prep: per-core input dicts (cheap numpy reshapes)."""
    f32 = np.float32
    scale = 1.0 / np.sqrt(DK)
    xT = {}
    for b in range(B):
        xT[b] = (
            np.ascontiguousarray(q[b].T).astype(BF16NP),
            np.ascontiguousarray(k[b].T).astype(BF16NP),
            np.ascontiguousarray(v[b].T).astype(BF16NP),
        )
    per_g = {}
    for g in range(2):
        F = slice(512 * g, 512 * g + 512)
        per_g[g] = dict(
            wq=np.ascontiguousarray(Wq[F].T * scale).astype(BF16NP),
            wk=np.ascontiguousarray(Wk[F].T).astype(BF16NP),
            wv=np.ascontiguousarray(Wv[F].T).astype(BF16NP),
            wo=np.ascontiguousarray(Wo[:, F].T).astype(BF16NP),
            bq=np.ascontiguousarray(
                (bq[F] * scale).reshape(4, 128).T, dtype=f32
            ),
        )
    in_maps = []
    for c in range(N_CORES):
        b, g = c // 2, c % 2
        qb, kb, vb = xT[b]
        in_maps.append(dict(qT=qb, kT=kb, vT=vb, **per_g[g]))
    return in_maps


def kernel(q, k, v, Wq, bq, Wk, bk, Wv, bv, Wo, bo):
    q, k, v = (np.asarray(x, np.float32) for x in (q, k, v))
    Wq, bq, Wk, bk = (np.asarray(x, np.float32) for x in (Wq, bq, Wk, bk))
    Wv, bv, Wo, bo = (np.asarray(x, np.float32) for x in (Wv, bv, Wo, bo))

    nc = get_nc(S)
    in_maps = _prep_in_maps(q, k, v, Wq, bq, Wk, Wv, Wo)
    res = run_bass_kernel_spmd(nc, in_maps, core_ids=list(range(N_CORES)))

    # bk drops out of softmax; bv folds into an effective output bias.
    bo_eff = (
        bo.astype(np.float64) + Wo.astype(np.float64) @ bv.astype(np.float64)
    ).astype(np.float32)
    out = np.empty((B, S, D), np.float32)
    for b in range(B):
        acc = res.results[2 * b]["outT"] + res.results[2 * b + 1]["outT"]
        out[b] = acc.T + bo_eff
    return out
